# revision 2
# baseline (speedup 1.0000x reference)
"""BiAttention Trainium2 kernel.

Reference math (per batch; n = m = 1024, d = 512):
    sim[n,m] = (x1*w3) @ x2.T + s1[n] + s2[m] + bias,  s1 = x1@w1, s2 = x2@w2
    row softmax over m with x2-masked columns dropped -> attn_a = P_row @ x2
    col softmax over n with x1-masked rows dropped    -> q2c = P_col.T @ x1
    attn_b = P_row @ q2c

Mask compaction (host-side, exact): ~half the x2 columns are masked and
contribute exactly-zero row-softmax weight, so the m axis is gathered down
to the unmasked positions (padded to Mp = mp_kt*128 with lane2 = NEG so the
pad rows of ET are exactly 0).  Similarly the n axis is PERMUTED so the
unmasked x1 rows come first: the q2c contraction (whose x1 operand is
zeroed at masked rows) then only needs the first nq_kt k-tiles.  Outputs
come back n-permuted; the host applies the inverse permutation.

Kernel formulation (softmax is shift-invariant, so each direction only needs
the logit terms that vary along its own axis):
    ET[m',n] = exp(s3c[m',n] + lane2c[m']),  lane2c = s2[idx2] + bias
        (s1[n] cancels in the row softmax; lane2c is per-partition in the
         [m',n] layout -> applied as the ACT exp bias; pads get NEG -> 0)
    TC = ET^T  [n-part, m'-free] (PE transpose)
    rowsum[n]  = sum_m' TC[n,m']        (DVE reduce; row scale = 1/rowsum)
    colsum'[m'] = sum_n ET[m',n]*BV[n],  BV = exp(s1)*keep1  (n-permuted;
        only the first nq_kt*128 columns can be nonzero)
    attn_a = (ET.T @ x2c) / rowsum
    q2c    = (TC.T @ (keep1*exp(s1)*x1)) / colsum'   (numerator and colsum'
        both carry the exp(lane2) factor -> plain reciprocal)
    attn_b = (ET.T @ q2c) / rowsum

Implementation notes:
  - exp() without max-subtraction: logits are O(+-8) here, and masking is
    additive -30000 so exp underflows to exactly 0.
  - Matmuls run in fp16 (10-bit mantissa; 2-byte operands stream at 1
    cycle/row with fast-weight-load). PSUM accumulation is fp32.
  - All gathers/permutes/casts are prepared host-side (pure layout
    marshaling); all O(n*m*d) compute runs on device.
  - Sharding: data-parallel over batch, 2 batches per core, 8 cores.
"""

import os
import sys

import numpy as np

for _p in ("/opt/trn_rl_repo",):
    if _p not in sys.path:
        sys.path.append(_p)

import concourse.bass as bass
import concourse.mybir as mybir
import concourse.tile as tile
from concourse import bass_utils
from concourse.bass import ds, ts
from concourse.tile import ScopedClock

NCORES = 8
B, N, M, D = 16, 1024, 1024, 512
BPC = B // NCORES  # batches per core
NEG = -30000.0  # additive mask: exp(x + NEG) == 0 for |x| < ~100

F32 = mybir.dt.float32
F16 = mybir.dt.float16

MM_DT = F16
MM_NP = np.float16

NT = N // 128  # 8 n-tiles
DC = D // 128  # 4 d-chunks
NH = N // 512  # 2 n-halves (PSUM-bank-sized slabs)

# ---------------------------------------------------------------------------
# Workarounds for this walrus build: at most ONE sync wait per instruction.
# ---------------------------------------------------------------------------

_ctr = [0]


def _split_multi_waits(nc):
    """Move extra sync waits onto same-engine InstNoOp carriers inserted
    immediately before the over-subscribed instruction."""
    for f in nc.m.functions:
        for bb in f.blocks:
            insts = bb.instructions
            i = 0
            while i < len(insts):
                inst = insts[i]
                si = getattr(inst, "sync_info", None)
                if si is not None and len(si.on_wait) > 1:
                    waits = list(si.on_wait)
                    carriers = []
                    for w in waits[:-1]:
                        _ctr[0] += 1
                        carriers.append(
                            mybir.InstNoOp(
                                name=f"I-waitsplit-{_ctr[0]}",
                                engine=inst.engine,
                                bass_nofuse=True,
                                sync_info=mybir.SyncInfo(on_wait=[w], on_update=[]),
                            )
                        )
                    inst.sync_info = mybir.SyncInfo(
                        on_wait=[waits[-1]], on_update=list(si.on_update)
                    )
                    insts[i:i] = carriers
                    i += len(carriers)
                i += 1


def _patched_drain_and_barrier(self, tick_clock, wait_clock):
    """TileContext tail drain: carry the global-clock waits on SP nops (the
    Drain opcode can't encode sync waits in this walrus build)."""
    nc = self.nc
    nop_inst = nc.sync.nop(nofuse=True)
    wait_clock.add_sem_waits(nop_inst.ins, ScopedClock({None: tick_clock.global_clock}))
    waits = list(nop_inst.ins.sync_info.on_wait)
    if len(waits) > 1:
        nop_inst.ins.sync_info = mybir.SyncInfo(on_wait=[waits[0]], on_update=[])
        for w in waits[1:]:
            extra = nc.sync.nop(nofuse=True)
            extra.ins.sync_info = mybir.SyncInfo(on_wait=[w], on_update=[])
    nc.sync.drain()
    nc.all_engine_barrier()
    assert self.sems is not None
    popped = nc._tile_sem_poison_stack.pop()
    assert popped is self._sem_poison
    nc.clear_and_free_semaphores(list(self.sems.allocated().values()))


tile.TileContext._drain_and_barrier = _patched_drain_and_barrier

# ---------------------------------------------------------------------------
# Kernel build
# ---------------------------------------------------------------------------

_cache = {}


def _build(mp_kt, nq_kt):
    MP = 128 * mp_kt  # compacted (padded) m size
    NQ = 128 * nq_kt  # n-tiles carrying all unmasked x1 rows (permuted)

    nc = bass.Bass("TRN2", target_bir_lowering=False, debug=False)

    # transposed operands for the similarity matmuls (d on partitions)
    x1td = nc.dram_tensor("x1t", [BPC, D, N], MM_DT, kind="ExternalInput").ap()
    x2td = nc.dram_tensor("x2t", [BPC, D, MP], MM_DT, kind="ExternalInput").ap()
    # natural-layout rhs operands
    x1md = nc.dram_tensor("x1m", [BPC, NQ, D], MM_DT, kind="ExternalInput").ap()
    x2d = nc.dram_tensor("x2", [BPC, MP, D], MM_DT, kind="ExternalInput").ap()
    # per-m' exp bias lane2c, [128, mp_kt] per-partition layout (fp32)
    lvecd = nc.dram_tensor("lvec", [BPC, 128, mp_kt], F32, kind="ExternalInput").ap()
    # broadcast row source (fp16): exp(s1)*keep1, n-permuted, first NQ entries
    bcd = nc.dram_tensor("bc", [BPC, NQ], MM_DT, kind="ExternalInput").ap()
    idd = nc.dram_tensor("ident", [128, 128], MM_DT, kind="ExternalInput").ap()
    oad = nc.dram_tensor("attn_a", [BPC, N, D], F32, kind="ExternalOutput").ap()
    obd = nc.dram_tensor("attn_b", [BPC, N, D], F32, kind="ExternalOutput").ap()

    EXP = mybir.ActivationFunctionType.Exp
    AX = mybir.AxisListType.X

    with tile.TileContext(nc) as tc:
        with (
            tc.tile_pool(name="xin", bufs=2) as xin,
            tc.tile_pool(name="amat", bufs=2) as amat,
            tc.tile_pool(name="emat", bufs=2) as emat,
            tc.tile_pool(name="qmat", bufs=2) as qmat,
            tc.tile_pool(name="small", bufs=2) as small,
            tc.tile_pool(name="tmp", bufs=3) as tmp,
            tc.tile_pool(name="ostage", bufs=4) as ostage,
            tc.tile_pool(name="mm_ps", bufs=4, space="PSUM") as mm_ps,
            tc.tile_pool(name="acc_ps", bufs=3, space="PSUM") as acc_ps,
        ):
            # trigger the ACT exp table load while the first DMAs are in
            # flight (the first real exp would otherwise pay ~2.7us mid-loop)
            warm = small.tile([128, 2], F32, tag="warm")
            nc.vector.memset(warm[:], 0.0)
            nc.scalar.activation(out=warm[:], in_=warm[:], func=EXP)
            # warm the PE clock (HAM) with dummy matmuls during the load
            # wait; without this the first ~3.4us of real matmuls run at
            # half clock
            wsb = small.tile([128, 512], MM_DT, tag="wsb")
            nc.vector.memset(wsb[:], 0.0)
            wps = mm_ps.tile([128, 512], F32, tag="mm")
            for _ in range(16):
                nc.tensor.matmul(
                    wps[:], wsb[:, 0:128], wsb[:], start=True, stop=True
                )
            ident = small.tile([128, 128], MM_DT, tag="ident")
            nc.sync.dma_start(out=ident[:], in_=idd)

            for b in range(BPC):
                # ---- loads ------------------------------------------------
                lvec = small.tile([128, mp_kt], F32, tag="lvec")
                nc.sync.dma_start(out=lvec[:], in_=lvecd[b])
                # A1/A2 feed the first matmuls: split the loads across the
                # sync and gpsimd queues so triggers issue in parallel
                A1 = amat.tile([128, DC, N], MM_DT, tag="A1")  # w3*x1^T (n-perm)
                A2 = amat.tile([128, DC, MP], MM_DT, tag="A2")  # x2^T compacted
                nc.sync.dma_start(
                    out=A2[:], in_=x2td[b].rearrange("(c p) m -> p c m", p=128)
                )
                nc.gpsimd.dma_start(
                    out=A1[:, :, 0:512],
                    in_=x1td[b][:, 0:512].rearrange("(c p) n -> p c n", p=128),
                )
                nc.gpsimd.dma_start(
                    out=A1[:, :, 512:1024],
                    in_=x1td[b][:, 512:1024].rearrange("(c p) n -> p c n", p=128),
                )
                BV = small.tile([128, NQ], MM_DT, tag="BV")
                nc.gpsimd.dma_start(
                    out=BV[:], in_=bcd[b][None, :].to_broadcast([128, NQ])
                )
                X1M = xin.tile([128, nq_kt, D], MM_DT, tag="X1M")
                X2 = xin.tile([128, mp_kt, D], MM_DT, tag="X2")
                nc.gpsimd.dma_start(
                    out=X1M[:], in_=x1md[b].rearrange("(t p) d -> p t d", p=128)
                )
                nc.gpsimd.dma_start(
                    out=X2[:], in_=x2d[b].rearrange("(t p) d -> p t d", p=128)
                )

                # ---- ET = exp(s3c^T + lane2c[m'])  [m'-part, n-free] ------
                ET = emat.tile([128, mp_kt, N], MM_DT, tag="ET")
                for nh in range(NH):
                    for mt in range(mp_kt):
                        ps = mm_ps.tile([128, 512], F32, tag="mm")
                        for c in range(DC):
                            nc.tensor.matmul(
                                ps[:],
                                A2[:, c, ts(mt, 128)],
                                A1[:, c, ds(512 * nh, 512)],
                                start=(c == 0),
                                stop=(c == DC - 1),
                            )
                        nc.scalar.activation(
                            out=ET[:, mt, ds(512 * nh, 512)],
                            in_=ps[:],
                            func=EXP,
                            bias=lvec[:, mt : mt + 1],
                        )
                # ---- TC = ET^T  [n-part, m'-free] (PE transpose) ----------
                TC = emat.tile([128, NT, MP], MM_DT, tag="TC")
                for nt in range(NT):
                    tps = mm_ps.tile([128, MP], MM_DT, tag="mm")
                    for mt in range(mp_kt):
                        nc.tensor.transpose(
                            tps[:, ts(mt, 128)],
                            ET[:, mt, ts(nt, 128)],
                            ident[:],
                        )
                    nc.vector.tensor_copy(TC[:, nt, :], tps[:])

                # ---- denominators off the PE ------------------------------
                # attn numerators and rowsum share the exp(s1[n]) factor, so
                # it cancels: the row scale is simply 1 / sum_m' TC[n,m']
                RR = small.tile([128, NT], F32, tag="RR")
                rtmp = small.tile([128, NT], F32, tag="rtmp")
                for nt in range(NT):
                    nc.vector.reduce_sum(
                        out=rtmp[:, nt : nt + 1], in_=TC[:, nt, :], axis=AX
                    )
                    nc.vector.reciprocal(RR[:, nt : nt + 1], rtmp[:, nt : nt + 1])
                # q2c scale[m'] = 1 / (sum_n ET[m',n]*BV[n] + eps); BV is
                # nonzero only in the first NQ (permuted) columns
                CR = small.tile([128, mp_kt], F32, tag="CR")
                ctmp = small.tile([128, mp_kt], F32, tag="ctmp")
                for mc in range(mp_kt):
                    scr = tmp.tile([128, NQ], MM_DT, tag="scr")
                    nc.vector.tensor_mul(scr[:], ET[:, mc, 0:NQ], BV[:])
                    nc.vector.reduce_sum(out=ctmp[:, mc : mc + 1], in_=scr[:], axis=AX)
                    nc.vector.tensor_scalar_add(
                        ctmp[:, mc : mc + 1], ctmp[:, mc : mc + 1], 1e-30
                    )
                    # numerator and colsum' both carry exp(lane2c[m'])
                    # -> plain reciprocal
                    nc.vector.reciprocal(CR[:, mc : mc + 1], ctmp[:, mc : mc + 1])

                # ---- attn_a = (ET.T @ x2c) / rowsum -----------------------
                for nt in range(NT):
                    aps = acc_ps.tile([128, 512], F32, tag="acc")
                    for mc in range(mp_kt):
                        nc.tensor.matmul(
                            aps[:],
                            ET[:, mc, ts(nt, 128)],
                            X2[:, mc, :],
                            start=(mc == 0),
                            stop=(mc == mp_kt - 1),
                        )
                    stage = ostage.tile([128, 512], F32, tag="stage")
                    nc.scalar.mul(stage[:], aps[:], RR[:, nt : nt + 1])
                    nc.sync.dma_start(out=oad[b, ts(nt, 128), :], in_=stage[:])

                # ---- q2c = (TC.T @ (keep1*exp(s1)*x1)) * CR ---------------
                Q2C = qmat.tile([128, mp_kt, D], MM_DT, tag="Q2C")
                for mt in range(mp_kt):
                    qps = acc_ps.tile([128, 512], F32, tag="acc")
                    for nq in range(nq_kt):
                        nc.tensor.matmul(
                            qps[:],
                            TC[:, nq, ts(mt, 128)],
                            X1M[:, nq, :],
                            start=(nq == 0),
                            stop=(nq == nq_kt - 1),
                        )
                    nc.scalar.mul(Q2C[:, mt, :], qps[:], CR[:, mt : mt + 1])

                # ---- attn_b = (ET.T @ q2c) / rowsum -----------------------
                for nt in range(NT):
                    bps = acc_ps.tile([128, 512], F32, tag="acc")
                    for mc in range(mp_kt):
                        nc.tensor.matmul(
                            bps[:],
                            ET[:, mc, ts(nt, 128)],
                            Q2C[:, mc, :],
                            start=(mc == 0),
                            stop=(mc == mp_kt - 1),
                        )
                    stage = ostage.tile([128, 512], F32, tag="stage")
                    nc.scalar.mul(stage[:], bps[:], RR[:, nt : nt + 1])
                    nc.sync.dma_start(out=obd[b, ts(nt, 128), :], in_=stage[:])

    _split_multi_waits(nc)
    return nc


def _get_nc(mp_kt, nq_kt):
    key = (mp_kt, nq_kt)
    if key not in _cache:
        _cache[key] = _build(mp_kt, nq_kt)
    return _cache[key]


# ---------------------------------------------------------------------------
# Host entry point
# ---------------------------------------------------------------------------


def _prep(x1, x1_mask, x2, x2_mask, w, bias):
    """Host-side marshaling: mask compaction (m gather, n permute), layout
    transposes, fp16 casts, and the tiny O(b*(n+m)) logit vectors."""
    x1 = np.asarray(x1, dtype=np.float32)
    x2 = np.asarray(x2, dtype=np.float32)
    x1_mask = np.asarray(x1_mask, dtype=bool)
    x2_mask = np.asarray(x2_mask, dtype=bool)
    w = np.asarray(w, dtype=np.float32)
    bias_f = float(np.asarray(bias, dtype=np.float32))

    b_sz, n, d = x1.shape
    w1, w2, w3 = w[:d], w[d : 2 * d], w[2 * d :]
    s1 = np.einsum("bnd,d->bn", x1, w1)
    s2 = np.einsum("bmd,d->bm", x2, w2)
    keep1 = np.where(x1_mask, np.float32(0), np.float32(1))
    es1v = np.exp(s1)

    k2 = (~x2_mask).sum(axis=1)
    n1 = (~x1_mask).sum(axis=1)
    mp_kt = max(1, int(-(-int(k2.max()) // 128)))
    nq_kt = max(1, int(-(-int(n1.max()) // 128)))
    MP, NQ = 128 * mp_kt, 128 * nq_kt

    x1t = np.zeros((b_sz, d, n), MM_NP)
    x2t = np.zeros((b_sz, d, MP), MM_NP)
    x1m = np.zeros((b_sz, NQ, d), MM_NP)
    x2c = np.zeros((b_sz, MP, d), MM_NP)
    lvec = np.full((b_sz, 128, mp_kt), NEG, np.float32)
    bc = np.zeros((b_sz, NQ), MM_NP)
    inv_perm = np.zeros((b_sz, n), np.int64)

    for bi in range(b_sz):
        idx2 = np.nonzero(~x2_mask[bi])[0]
        kk = len(idx2)
        perm = np.argsort(x1_mask[bi], kind="stable")  # unmasked n first
        inv_perm[bi][perm] = np.arange(n)

        x1t[bi] = ((x1[bi] * w3)[perm].T).astype(MM_NP)
        x2t[bi, :, :kk] = (x2[bi][idx2].T).astype(MM_NP)
        x1mf = x1[bi] * (keep1[bi] * es1v[bi])[:, None]
        x1m[bi] = (x1mf[perm][:NQ]).astype(MM_NP)
        x2c[bi, :kk] = x2[bi][idx2].astype(MM_NP)
        lane2c = s2[bi][idx2] + bias_f
        # [MP] -> [128, mp_kt]: value for m'=t*128+p at [p, t]
        lv = np.full(MP, NEG, np.float32)
        lv[:kk] = lane2c
        lvec[bi] = lv.reshape(mp_kt, 128).T
        bc[bi] = ((keep1[bi] * es1v[bi])[perm][:NQ]).astype(MM_NP)

    return (
        {
            "x1t": np.ascontiguousarray(x1t),
            "x2t": np.ascontiguousarray(x2t),
            "x1m": np.ascontiguousarray(x1m),
            "x2": np.ascontiguousarray(x2c),
            "lvec": np.ascontiguousarray(lvec),
            "bc": np.ascontiguousarray(bc),
        },
        np.eye(128, dtype=MM_NP),
        inv_perm,
        mp_kt,
        nq_kt,
    )


def _run(x1, x1_mask, x2, x2_mask, w, bias, **run_kwargs):
    full, ident, inv_perm, mp_kt, nq_kt = _prep(
        x1, x1_mask, x2, x2_mask, w, bias
    )
    nc = _get_nc(mp_kt, nq_kt)
    in_maps = []
    for core in range(NCORES):
        lo, hi = core * BPC, (core + 1) * BPC
        m = {k: v[lo:hi] for k, v in full.items()}
        m["ident"] = ident
        in_maps.append(m)
    res = bass_utils.run_bass_kernel_spmd(
        nc, in_maps, core_ids=list(range(NCORES)), **run_kwargs
    )
    attn_a = np.concatenate([res.results[c]["attn_a"] for c in range(NCORES)], axis=0)
    attn_b = np.concatenate([res.results[c]["attn_b"] for c in range(NCORES)], axis=0)
    # undo the n permutation
    bidx = np.arange(B)[:, None]
    attn_a = attn_a[bidx, inv_perm]
    attn_b = attn_b[bidx, inv_perm]
    return (attn_a, attn_b), res


def kernel(x1, x1_mask, x2, x2_mask, w, bias):
    out, _ = _run(x1, x1_mask, x2, x2_mask, w, bias)
    return out


# revision 6
# speedup vs baseline: 1.2348x; 1.2348x over previous
"""BiAttention Trainium2 kernel.

Reference math (per batch; n = m = 1024, d = 512):
    sim[n,m] = (x1*w3) @ x2.T + s1[n] + s2[m] + bias,  s1 = x1@w1, s2 = x2@w2
    row softmax over m with x2-masked columns dropped -> attn_a = P_row @ x2
    col softmax over n with x1-masked rows dropped    -> q2c = P_col.T @ x1
    attn_b = P_row @ q2c

Mask compaction (host-side, exact): ~half the x2 columns are masked and
contribute exactly-zero row-softmax weight, so the m axis is gathered down
to the unmasked positions (padded to Mp = mp_kt*128 with lane2 = NEG so the
pad rows of ET are exactly 0).  Similarly the n axis is PERMUTED so the
unmasked x1 rows come first: the q2c contraction (whose x1 operand is
zeroed at masked rows) then only needs the first nq_kt k-tiles.  Outputs
come back n-permuted; the host applies the inverse permutation.

Kernel formulation (softmax is shift-invariant, so each direction only needs
the logit terms that vary along its own axis):
    ET[m',n] = exp(s3c[m',n] + lane2c[m']),  lane2c = s2[idx2] + bias
        (s1[n] cancels in the row softmax; lane2c is per-partition in the
         [m',n] layout -> applied as the ACT exp bias; pads get NEG -> 0)
    TC = ET^T  [n-part, m'-free] (PE transpose)
    rowsum[n]  = sum_m' TC[n,m']        (DVE reduce; row scale = 1/rowsum)
    colsum'[m'] = sum_n ET[m',n]*BV[n],  BV = exp(s1)*keep1  (n-permuted;
        only the first nq_kt*128 columns can be nonzero)
    attn_a = (ET.T @ x2c) / rowsum
    q2c    = (TC.T @ (keep1*exp(s1)*x1)) / colsum'   (numerator and colsum'
        both carry the exp(lane2) factor -> plain reciprocal)
    attn_b = (ET.T @ q2c) / rowsum

Implementation notes:
  - exp() without max-subtraction: logits are O(+-8) here, and masking is
    additive -30000 so exp underflows to exactly 0.
  - Matmuls run in fp16 (10-bit mantissa; 2-byte operands stream at 1
    cycle/row with fast-weight-load). PSUM accumulation is fp32.
  - All gathers/permutes/casts are prepared host-side (pure layout
    marshaling); all O(n*m*d) compute runs on device.
  - Sharding: data-parallel over batch, 2 batches per core, 8 cores.
"""

import os
import sys

import numpy as np

for _p in ("/opt/trn_rl_repo",):
    if _p not in sys.path:
        sys.path.append(_p)

import concourse.bass as bass
import concourse.mybir as mybir
import concourse.tile as tile
from concourse import bass_utils
from concourse.bass import ds, ts
from concourse.tile import ScopedClock

NCORES = 8
B, N, M, D = 16, 1024, 1024, 512
BPC = B // NCORES  # batches per core
NEG = -30000.0  # additive mask: exp(x + NEG) == 0 for |x| < ~100

F32 = mybir.dt.float32
F16 = mybir.dt.float16
E4 = mybir.dt.float8e4

MM_DT = F16
MM_NP = np.float16

NT = N // 128  # 8 n-tiles
DC = D // 128  # 4 d-chunks
NH = N // 512  # 2 n-halves (PSUM-bank-sized slabs)

# ---------------------------------------------------------------------------
# Workarounds for this walrus build: at most ONE sync wait per instruction.
# ---------------------------------------------------------------------------

_ctr = [0]


def _split_multi_waits(nc):
    """Move extra sync waits onto same-engine InstNoOp carriers inserted
    immediately before the over-subscribed instruction."""
    for f in nc.m.functions:
        for bb in f.blocks:
            insts = bb.instructions
            i = 0
            while i < len(insts):
                inst = insts[i]
                si = getattr(inst, "sync_info", None)
                if si is not None and len(si.on_wait) > 1:
                    waits = list(si.on_wait)
                    carriers = []
                    for w in waits[:-1]:
                        _ctr[0] += 1
                        carriers.append(
                            mybir.InstNoOp(
                                name=f"I-waitsplit-{_ctr[0]}",
                                engine=inst.engine,
                                bass_nofuse=True,
                                sync_info=mybir.SyncInfo(on_wait=[w], on_update=[]),
                            )
                        )
                    inst.sync_info = mybir.SyncInfo(
                        on_wait=[waits[-1]], on_update=list(si.on_update)
                    )
                    insts[i:i] = carriers
                    i += len(carriers)
                i += 1


def _patched_drain_and_barrier(self, tick_clock, wait_clock):
    """TileContext tail drain: carry the global-clock waits on SP nops (the
    Drain opcode can't encode sync waits in this walrus build)."""
    nc = self.nc
    nop_inst = nc.sync.nop(nofuse=True)
    wait_clock.add_sem_waits(nop_inst.ins, ScopedClock({None: tick_clock.global_clock}))
    waits = list(nop_inst.ins.sync_info.on_wait)
    if len(waits) > 1:
        nop_inst.ins.sync_info = mybir.SyncInfo(on_wait=[waits[0]], on_update=[])
        for w in waits[1:]:
            extra = nc.sync.nop(nofuse=True)
            extra.ins.sync_info = mybir.SyncInfo(on_wait=[w], on_update=[])
    nc.sync.drain()
    nc.all_engine_barrier()
    assert self.sems is not None
    popped = nc._tile_sem_poison_stack.pop()
    assert popped is self._sem_poison
    nc.clear_and_free_semaphores(list(self.sems.allocated().values()))


tile.TileContext._drain_and_barrier = _patched_drain_and_barrier

# ---------------------------------------------------------------------------
# Kernel build
# ---------------------------------------------------------------------------

_cache = {}


def _build(mp_kt, nq_kt):
    MP = 128 * mp_kt  # compacted (padded) m size
    NQ = 128 * nq_kt  # n-tiles carrying all unmasked x1 rows (permuted)

    nc = bass.Bass("TRN2", target_bir_lowering=False, debug=False)

    # transposed operands for the similarity matmuls (d on partitions)
    x1td = nc.dram_tensor("x1t", [BPC, D, N], MM_DT, kind="ExternalInput").ap()
    x2td = nc.dram_tensor("x2t", [BPC, D, MP], MM_DT, kind="ExternalInput").ap()
    # natural-layout rhs operands
    x1md = nc.dram_tensor("x1m", [BPC, NQ, D], MM_DT, kind="ExternalInput").ap()
    x2d = nc.dram_tensor("x2", [BPC, MP, D], MM_DT, kind="ExternalInput").ap()
    # per-m' exp bias lane2c, [128, mp_kt] per-partition layout (fp32)
    lvecd = nc.dram_tensor("lvec", [BPC, 128, mp_kt], F32, kind="ExternalInput").ap()
    # broadcast row source (fp16): exp(s1)*keep1, n-permuted, first NQ entries
    bcd = nc.dram_tensor("bc", [BPC, NQ], MM_DT, kind="ExternalInput").ap()
    idd = nc.dram_tensor("ident", [128, 128], MM_DT, kind="ExternalInput").ap()
    oad = nc.dram_tensor("attn_a", [BPC, N, D], F32, kind="ExternalOutput").ap()
    obd = nc.dram_tensor("attn_b", [BPC, N, D], F32, kind="ExternalOutput").ap()

    EXP = mybir.ActivationFunctionType.Exp
    AX = mybir.AxisListType.X

    with tile.TileContext(nc) as tc:
        with (
            tc.tile_pool(name="xin", bufs=2) as xin,
            tc.tile_pool(name="amat", bufs=2) as amat,
            tc.tile_pool(name="emat", bufs=2) as emat,
            tc.tile_pool(name="qmat", bufs=2) as qmat,
            tc.tile_pool(name="small", bufs=2) as small,
            tc.tile_pool(name="tmp", bufs=3) as tmp,
            tc.tile_pool(name="ostage", bufs=4) as ostage,
            tc.tile_pool(name="mm_ps", bufs=4, space="PSUM") as mm_ps,
            tc.tile_pool(name="acc_ps", bufs=3, space="PSUM") as acc_ps,
        ):
            # trigger the ACT exp table load while the first DMAs are in
            # flight (the first real exp would otherwise pay ~2.7us mid-loop)
            warm = small.tile([128, 2], F32, tag="warm")
            nc.vector.memset(warm[:], 0.0)
            nc.scalar.activation(out=warm[:], in_=warm[:], func=EXP)
            # warm the PE clock (HAM) with dummy matmuls during the load
            # wait; without this the first ~3us of real matmuls run at
            # half clock.  A few big ones then cheap 64-row fillers that
            # keep the PE continuously busy until the first loads land.
            wsb = small.tile([128, 512], MM_DT, tag="wsb")
            nc.vector.memset(wsb[:], 0.0)
            wps = mm_ps.tile([128, 512], F32, tag="mm")
            for _ in range(6):
                nc.tensor.matmul(
                    wps[:], wsb[:, 0:128], wsb[:], start=True, stop=True
                )
            for _ in range(24):
                nc.tensor.matmul(
                    wps[:, 0:64], wsb[:, 0:128], wsb[:, 0:64], start=True, stop=True
                )
            ident = small.tile([128, 128], MM_DT, tag="ident")
            nc.sync.dma_start(out=ident[:], in_=idd)

            for b in range(BPC):
                # ---- loads ------------------------------------------------
                lvec = small.tile([128, mp_kt], F32, tag="lvec")
                nc.sync.dma_start(out=lvec[:], in_=lvecd[b])
                # A1/A2 feed the first matmuls: chunk the leading loads by
                # d-chunk across the sync and gpsimd queues so the first
                # S matmul can start after ~0.3MB instead of ~1.1MB
                A1 = amat.tile([128, DC, N], MM_DT, tag="A1")  # w3*x1^T (n-perm)
                A2 = amat.tile([128, DC, MP], MM_DT, tag="A2")  # x2^T compacted
                for c in range(DC):
                    nc.sync.dma_start(
                        out=A2[:, c, :], in_=x2td[b][ds(128 * c, 128)]
                    )
                    nc.gpsimd.dma_start(
                        out=A1[:, c, 0:512],
                        in_=x1td[b][ds(128 * c, 128), 0:512],
                    )
                nc.gpsimd.dma_start(
                    out=A1[:, :, 512:1024],
                    in_=x1td[b][:, 512:1024].rearrange("(c p) n -> p c n", p=128),
                )
                BV = small.tile([128, NQ], MM_DT, tag="BV")
                nc.gpsimd.dma_start(
                    out=BV[:], in_=bcd[b][None, :].to_broadcast([128, NQ])
                )
                X1M = xin.tile([128, nq_kt, D], MM_DT, tag="X1M")
                X2 = xin.tile([128, mp_kt, D], MM_DT, tag="X2")
                nc.gpsimd.dma_start(
                    out=X1M[:], in_=x1md[b].rearrange("(t p) d -> p t d", p=128)
                )
                nc.gpsimd.dma_start(
                    out=X2[:], in_=x2d[b].rearrange("(t p) d -> p t d", p=128)
                )

                # ---- ET = exp(s3c^T + lane2c[m'])  [m'-part, n-free] ------
                # ET8 is the fp8 shadow for the attn_b matmuls (the row
                # softmax weights only; rel-err impact ~1e-2 on attn_b)
                ET = emat.tile([128, mp_kt, N], MM_DT, tag="ET")
                ET8 = emat.tile([128, mp_kt, N], E4, tag="ET8")
                for nh in range(NH):
                    for mt in range(mp_kt):
                        ps = mm_ps.tile([128, 512], F32, tag="mm")
                        for c in range(DC):
                            nc.tensor.matmul(
                                ps[:],
                                A2[:, c, ts(mt, 128)],
                                A1[:, c, ds(512 * nh, 512)],
                                start=(c == 0),
                                stop=(c == DC - 1),
                            )
                        nc.scalar.activation(
                            out=ET[:, mt, ds(512 * nh, 512)],
                            in_=ps[:],
                            func=EXP,
                            bias=lvec[:, mt : mt + 1],
                        )
                        nc.vector.tensor_copy(
                            ET8[:, mt, ds(512 * nh, 512)],
                            ET[:, mt, ds(512 * nh, 512)],
                        )
                # ---- TC = ET^T  [n-part, m'-free] (PE transpose) ----------
                # Only the first nq_kt n-tiles feed the q2c matmul; the rest
                # are transposed just for rowsum and reduced straight out of
                # PSUM without an SBUF copy.
                RR = small.tile([128, NT], F32, tag="RR")
                rtmp = small.tile([128, NT], F32, tag="rtmp")
                TC = emat.tile([128, nq_kt, MP], MM_DT, tag="TC")
                for nt in range(NT):
                    tps = mm_ps.tile([128, MP], MM_DT, tag="mm")
                    for mt in range(mp_kt):
                        nc.tensor.transpose(
                            tps[:, ts(mt, 128)],
                            ET[:, mt, ts(nt, 128)],
                            ident[:],
                        )
                    if nt < nq_kt:
                        nc.vector.tensor_copy(TC[:, nt, :], tps[:])
                        red_src = TC[:, nt, :]
                    else:
                        red_src = tps[:]
                    # attn numerators and rowsum share the exp(s1[n]) factor,
                    # so it cancels: the row scale is 1 / sum_m' ET^T[n,m']
                    nc.vector.reduce_sum(
                        out=rtmp[:, nt : nt + 1], in_=red_src, axis=AX
                    )
                    nc.vector.reciprocal(RR[:, nt : nt + 1], rtmp[:, nt : nt + 1])
                # q2c scale[m'] = 1 / (sum_n ET[m',n]*BV[n] + eps); BV is
                # nonzero only in the first NQ (permuted) columns
                CR = small.tile([128, mp_kt], F32, tag="CR")
                ctmp = small.tile([128, mp_kt], F32, tag="ctmp")
                for mc in range(mp_kt):
                    scr = tmp.tile([128, NQ], MM_DT, tag="scr")
                    nc.vector.tensor_mul(scr[:], ET[:, mc, 0:NQ], BV[:])
                    nc.vector.reduce_sum(out=ctmp[:, mc : mc + 1], in_=scr[:], axis=AX)
                    nc.vector.tensor_scalar_add(
                        ctmp[:, mc : mc + 1], ctmp[:, mc : mc + 1], 1e-30
                    )
                    # numerator and colsum' both carry exp(lane2c[m'])
                    # -> plain reciprocal
                    nc.vector.reciprocal(CR[:, mc : mc + 1], ctmp[:, mc : mc + 1])

                # ---- attn_a = (ET.T @ x2c) / rowsum -----------------------
                for nt in range(NT):
                    aps = acc_ps.tile([128, 512], F32, tag="acc")
                    for mc in range(mp_kt):
                        nc.tensor.matmul(
                            aps[:],
                            ET[:, mc, ts(nt, 128)],
                            X2[:, mc, :],
                            start=(mc == 0),
                            stop=(mc == mp_kt - 1),
                        )
                    stage = ostage.tile([128, 512], F32, tag="stage")
                    nc.scalar.mul(stage[:], aps[:], RR[:, nt : nt + 1])
                    nc.sync.dma_start(out=oad[b, ts(nt, 128), :], in_=stage[:])

                # ---- q2c = (TC.T @ (keep1*exp(s1)*x1)) * CR ---------------
                Q2C8 = qmat.tile([128, mp_kt, D], E4, tag="Q2C8")
                for mt in range(mp_kt):
                    qps = acc_ps.tile([128, 512], F32, tag="acc")
                    for nq in range(nq_kt):
                        nc.tensor.matmul(
                            qps[:],
                            TC[:, nq, ts(mt, 128)],
                            X1M[:, nq, :],
                            start=(nq == 0),
                            stop=(nq == nq_kt - 1),
                        )
                    nc.scalar.mul(Q2C8[:, mt, :], qps[:], CR[:, mt : mt + 1])

                # ---- attn_b = (ET.T @ q2c) / rowsum  (fp8 DoubleRow) ------
                ndr = mp_kt // 2
                for nt in range(NT):
                    bps = acc_ps.tile([128, 512], F32, tag="acc")
                    for j in range(ndr):
                        nc.tensor.matmul(
                            bps[:],
                            ET8[:, 2 * j : 2 * j + 2, ts(nt, 128)],
                            Q2C8[:, 2 * j : 2 * j + 2, :],
                            start=(j == 0),
                            stop=(j == ndr - 1 and mp_kt % 2 == 0),
                            perf_mode=mybir.MatmulPerfMode.DoubleRow,
                        )
                    if mp_kt % 2:
                        nc.tensor.matmul(
                            bps[:],
                            ET8[:, mp_kt - 1, ts(nt, 128)],
                            Q2C8[:, mp_kt - 1, :],
                            start=(mp_kt == 1),
                            stop=True,
                        )
                    stage = ostage.tile([128, 512], F32, tag="stage")
                    nc.scalar.mul(stage[:], bps[:], RR[:, nt : nt + 1])
                    nc.sync.dma_start(out=obd[b, ts(nt, 128), :], in_=stage[:])

    _split_multi_waits(nc)
    return nc


def _get_nc(mp_kt, nq_kt):
    key = (mp_kt, nq_kt)
    if key not in _cache:
        _cache[key] = _build(mp_kt, nq_kt)
    return _cache[key]


# ---------------------------------------------------------------------------
# Host entry point
# ---------------------------------------------------------------------------


def _prep(x1, x1_mask, x2, x2_mask, w, bias):
    """Host-side marshaling: mask compaction (m gather, n permute), layout
    transposes, fp16 casts, and the tiny O(b*(n+m)) logit vectors."""
    x1 = np.asarray(x1, dtype=np.float32)
    x2 = np.asarray(x2, dtype=np.float32)
    x1_mask = np.asarray(x1_mask, dtype=bool)
    x2_mask = np.asarray(x2_mask, dtype=bool)
    w = np.asarray(w, dtype=np.float32)
    bias_f = float(np.asarray(bias, dtype=np.float32))

    b_sz, n, d = x1.shape
    w1, w2, w3 = w[:d], w[d : 2 * d], w[2 * d :]
    s1 = np.einsum("bnd,d->bn", x1, w1)
    s2 = np.einsum("bmd,d->bm", x2, w2)
    keep1 = np.where(x1_mask, np.float32(0), np.float32(1))
    es1v = np.exp(s1)

    k2 = (~x2_mask).sum(axis=1)
    n1 = (~x1_mask).sum(axis=1)
    mp_kt = max(1, int(-(-int(k2.max()) // 128)))
    nq_kt = max(1, int(-(-int(n1.max()) // 128)))
    MP, NQ = 128 * mp_kt, 128 * nq_kt

    x1t = np.zeros((b_sz, d, n), MM_NP)
    x2t = np.zeros((b_sz, d, MP), MM_NP)
    x1m = np.zeros((b_sz, NQ, d), MM_NP)
    x2c = np.zeros((b_sz, MP, d), MM_NP)
    lvec = np.full((b_sz, 128, mp_kt), NEG, np.float32)
    bc = np.zeros((b_sz, NQ), MM_NP)
    inv_perm = np.zeros((b_sz, n), np.int64)

    for bi in range(b_sz):
        idx2 = np.nonzero(~x2_mask[bi])[0]
        kk = len(idx2)
        perm = np.argsort(x1_mask[bi], kind="stable")  # unmasked n first
        inv_perm[bi][perm] = np.arange(n)

        x1t[bi] = ((x1[bi] * w3)[perm].T).astype(MM_NP)
        x2t[bi, :, :kk] = (x2[bi][idx2].T).astype(MM_NP)
        x1mf = x1[bi] * (keep1[bi] * es1v[bi])[:, None]
        x1m[bi] = (x1mf[perm][:NQ]).astype(MM_NP)
        x2c[bi, :kk] = x2[bi][idx2].astype(MM_NP)
        lane2c = s2[bi][idx2] + bias_f
        # [MP] -> [128, mp_kt]: value for m'=t*128+p at [p, t]
        lv = np.full(MP, NEG, np.float32)
        lv[:kk] = lane2c
        lvec[bi] = lv.reshape(mp_kt, 128).T
        bc[bi] = ((keep1[bi] * es1v[bi])[perm][:NQ]).astype(MM_NP)

    return (
        {
            "x1t": np.ascontiguousarray(x1t),
            "x2t": np.ascontiguousarray(x2t),
            "x1m": np.ascontiguousarray(x1m),
            "x2": np.ascontiguousarray(x2c),
            "lvec": np.ascontiguousarray(lvec),
            "bc": np.ascontiguousarray(bc),
        },
        np.eye(128, dtype=MM_NP),
        inv_perm,
        mp_kt,
        nq_kt,
    )


def _run(x1, x1_mask, x2, x2_mask, w, bias, **run_kwargs):
    full, ident, inv_perm, mp_kt, nq_kt = _prep(
        x1, x1_mask, x2, x2_mask, w, bias
    )
    nc = _get_nc(mp_kt, nq_kt)
    in_maps = []
    for core in range(NCORES):
        lo, hi = core * BPC, (core + 1) * BPC
        m = {k: v[lo:hi] for k, v in full.items()}
        m["ident"] = ident
        in_maps.append(m)
    res = bass_utils.run_bass_kernel_spmd(
        nc, in_maps, core_ids=list(range(NCORES)), **run_kwargs
    )
    attn_a = np.concatenate([res.results[c]["attn_a"] for c in range(NCORES)], axis=0)
    attn_b = np.concatenate([res.results[c]["attn_b"] for c in range(NCORES)], axis=0)
    # undo the n permutation
    bidx = np.arange(B)[:, None]
    attn_a = attn_a[bidx, inv_perm]
    attn_b = attn_b[bidx, inv_perm]
    return (attn_a, attn_b), res


def kernel(x1, x1_mask, x2, x2_mask, w, bias):
    out, _ = _run(x1, x1_mask, x2, x2_mask, w, bias)
    return out
